# revision 32
# baseline (speedup 1.0000x reference)
"""Trainium2 Bass kernel for DiscreteGCNLayer.

Computation (per batch b):
    dw      = ternary_quantize(weight, s=0.01)            # [256, 256]
    support = x[b] @ dw                                   # [2048, 256]
    out[b]  = relu(adj[b] @ support + bias)               # [2048, 256]

Strategy: data-parallel over the batch dim (8 batches -> 8 NeuronCores),
weight/bias replicated.  The kernel is HBM-bandwidth dominated (adj is
16 MB/core in fp32), so the wire format is bf16: inputs are downconverted
on the host (tolerance 2e-2 >> bf16 rounding) and laid out so that every
matmul operand lands in SBUF already in lhsT orientation:

  xt[b]   = x[b].T                      [Din, N]  bf16  (stage-1 lhsT)
  adjt[b] = per-128-row-block transpose [NB, 128, N] bf16 with
            adjt[nb, p, c*128+j] = adj[nb*128+j, c*128+p]  (one fully
            contiguous 512 KB slab per row block, 4 KB per partition
            line -> full-rate DMA)

This removes all PE transposes and their PSUM->SBUF copy traffic.  Stage 2
is computed TRANSPOSED: outT[o, n] = sum_m support[m, o] * adjT[m, n], with
support chunks as lhsT (natural layout) and the adjt slabs as the moving
operand.  With o on the partition axis, the bias add is a per-partition
scalar that fuses into the relu eviction on DVE/ACT for free (no rank-1
bias matmuls), and the host un-transposes the bf16 output while upcasting.
The PE therefore executes only the two GEMMs' mathematically minimal
cycle count, plus a short warm-up burst that lifts the cold-clock
throttle while the first DMAs land.  DMA issue order is the schedule
(transfers serialize): weight -> x quarters (stage 1 consumes them
incrementally) -> adj row blocks, sized so the PE never starves.
"""

import sys

import numpy as np

if "/opt/trn_rl_repo" not in sys.path:
    sys.path.insert(0, "/opt/trn_rl_repo")

B = 8
N = 2048
DIN = 256
DOUT = 256
P = 128
NB = N // P  # 16 row blocks (stage-2 output)
MB = N // P  # 16 contraction chunks (stage 2)
IB = DIN // P  # 2 contraction chunks (stage 1)
OH = DOUT // P  # 2 output column halves (stage 2 psum partition groups)
SPARSITY = 0.01

_NC = None


def _build_nc():
    from contextlib import ExitStack

    import concourse.bass as bass  # noqa: F401  (registers engines)
    import concourse.mybir as mybir
    import concourse.tile as tile
    from concourse import bacc

    F32 = mybir.dt.float32
    BF16 = mybir.dt.bfloat16
    Alu = mybir.AluOpType

    nc = bacc.Bacc()
    xt_d = nc.dram_tensor("xt", [DIN, N], BF16, kind="ExternalInput")
    adjt_d = nc.dram_tensor("adjt", [NB, P, N], BF16, kind="ExternalInput")
    w_d = nc.dram_tensor("weight", [DIN, DOUT], F32, kind="ExternalInput")
    b_d = nc.dram_tensor("bias", [DOUT], F32, kind="ExternalInput")
    out_d = nc.dram_tensor("out", [DOUT, N], BF16, kind="ExternalOutput")

    with tile.TileContext(nc) as tc, ExitStack() as ctx:
        singles = ctx.enter_context(tc.tile_pool(name="singles", bufs=1))
        out_pool = ctx.enter_context(tc.tile_pool(name="outsb", bufs=2))
        psum_s1 = ctx.enter_context(tc.tile_pool(name="ps1", bufs=6, space="PSUM"))
        psum_s2 = ctx.enter_context(tc.tile_pool(name="ps2", bufs=2, space="PSUM"))

        # --- DMA kickoff.  All transfers serialize on the DMA engines, so
        # issue order is the schedule: weight first (quantization overlaps
        # the x stream), x quarters feed stage 1 incrementally, adj
        # row-block 0 lands just as stage 1 finishes, and the remaining adj
        # slabs (singles, then 1 MB pairs) stay ahead of the PE's
        # 1.7 us/row-block consumption.  Every DMA is kept >= 256 KB: the
        # ~650 ns HWDGE config per transfer otherwise paces the stream.
        w_sb = singles.tile([P, IB, DOUT], F32)
        nc.sync.dma_start(out=w_sb, in_=w_d[:].rearrange("(c p) o -> p c o", p=P))
        bias_sb = singles.tile([P, OH], F32)
        nc.gpsimd.dma_start(out=bias_sb, in_=b_d[:].rearrange("(c p) -> p c", p=P))

        xt_sb = singles.tile([P, IB, N], BF16)
        xt_r = xt_d[:].rearrange("(c p) m -> p c m", p=P)
        XQ = 4  # x quarters
        QW = N // XQ
        for q in range(XQ):
            nc.sync.dma_start(
                out=xt_sb[:, :, q * QW : (q + 1) * QW],
                in_=xt_r[:, :, q * QW : (q + 1) * QW],
            )

        adj_sb = singles.tile([P, NB, N], BF16)  # 64 KB/partition, all of adjt
        for nb in range(6):  # single row-block slabs keep the PE fed early
            nc.sync.dma_start(
                out=adj_sb[:, nb, :],
                in_=adjt_d[nb : nb + 1].rearrange("b p f -> p b f")[:, 0, :],
            )
        for g in range(5):  # 1 MB slabs covering nb = 6..15
            lo = 6 + 2 * g
            nc.sync.dma_start(
                out=adj_sb[:, lo : lo + 2, :],
                in_=adjt_d[lo : lo + 2].rearrange("b p f -> p b f"),
            )

        # --- PE warm-up burst: the clock needs ~3us of sustained PE
        # activity to leave the cold throttle; spend the DMA-bound startup
        # ramping on junk matmuls.
        junk = singles.tile([P, 512], BF16)
        nc.vector.memset(junk, 1.0)
        # tiny dummy activation: bacc places the ACT function-table load
        # before the first InstActivation in program order, so this hoists
        # the 1.3us LoadActFuncSet to t~0 instead of mid-kernel where it
        # head-of-line blocks the first PSUM evictions.
        actwarm = singles.tile([1, 8], BF16)
        nc.scalar.activation(
            actwarm, junk[0:1, 0:8], mybir.ActivationFunctionType.Relu
        )
        for wu in range(7):
            wt = psum_s2.tile([P, 512], F32, tag="s2")
            nc.tensor.matmul(wt, lhsT=junk[:, 0:P], rhs=junk, start=True, stop=True)

        # ternary-quantized weight in bf16: dw = ((w > s) - (w < -s)) * s
        # (per i-chunk so the first chunk is ready before the first x
        # quarter lands; DVE only -- GPSIMD cannot touch PSUM but these are
        # SBUF->SBUF, it is the eviction rotation that must avoid Pool)
        dw_sb = singles.tile([P, IB, DOUT], BF16)
        tpos = singles.tile([P, IB, DOUT], F32)
        tneg = singles.tile([P, IB, DOUT], F32)
        for c in range(IB):
            nc.vector.tensor_scalar(
                out=tpos[:, c, :], in0=w_sb[:, c, :], scalar1=SPARSITY,
                scalar2=SPARSITY, op0=Alu.is_gt, op1=Alu.mult,
            )
            nc.vector.tensor_scalar(
                out=tneg[:, c, :], in0=w_sb[:, c, :], scalar1=-SPARSITY,
                scalar2=SPARSITY, op0=Alu.is_lt, op1=Alu.mult,
            )
            nc.vector.tensor_sub(dw_sb[:, c, :], tpos[:, c, :], tneg[:, c, :])

        # --- stage 1: support[mb][p, o] = sum_i x[128*mb+p, i] dw[i, o]
        support = singles.tile([P, MB, DOUT], BF16)
        for mb in range(MB):
            sp = psum_s1.tile([P, DOUT], F32, tag="s1")
            for c in range(IB):
                nc.tensor.matmul(
                    sp,
                    lhsT=xt_sb[:, c, mb * P : (mb + 1) * P],
                    rhs=dw_sb[:, c, :],
                    start=(c == 0),
                    stop=(c == IB - 1),
                )
            if mb % 2 == 0:
                nc.scalar.copy(support[:, mb, :], sp)
            else:
                nc.vector.tensor_copy(support[:, mb, :], sp)

        # --- stage 2 (transposed): outT[oh][o, n-block nb] =
        #       relu( sum_c support[c][:, oh].T @ adjt[nb][c] + bias[oh] )
        # Evictions fuse the per-partition bias add + relu + bf16 downconvert
        # in one op, alternating DVE / ACT.  Stores: one big [nb 0..13] batch
        # per half (ready only after the last adj slab, so it cannot preempt
        # the load stream on the DMA device) plus a small final [14,15] store
        # whose two halves go out on SP and ACT in parallel; the very last
        # chain is nb15-oh0 so the tail is the fast DVE-evict -> SP-store path.
        GRPS = [(0, 14), (14, 2)]
        grp_of = {}
        for g in GRPS:
            for nb in range(g[0], g[0] + g[1]):
                grp_of[nb] = g
        osb = [None, None]
        for nb in range(NB):
            g0, gl = grp_of[nb]
            if nb == g0:
                osb[0] = out_pool.tile(
                    [P, gl * P], BF16, tag="o0", name=f"osb0_{nb}", bufs=2
                )
                osb[1] = out_pool.tile(
                    [P, gl * P], BF16, tag="o1", name=f"osb1_{nb}", bufs=2
                )
            oh_order = (1, 0) if nb == NB - 1 else (0, 1)
            for oh in oh_order:
                op = psum_s2.tile([P, P], F32, tag="s2")
                for c in range(MB):
                    nc.tensor.matmul(
                        op,
                        lhsT=support[:, c, oh * P : (oh + 1) * P],
                        rhs=adj_sb[:, nb, c * P : (c + 1) * P],
                        start=(c == 0),
                        stop=(c == MB - 1),
                    )
                dst = osb[oh][:, (nb - g0) * P : (nb - g0 + 1) * P]
                if oh == 0:
                    nc.vector.tensor_scalar(
                        out=dst, in0=op, scalar1=bias_sb[:, 0:1], scalar2=0.0,
                        op0=Alu.add, op1=Alu.max,
                    )
                else:
                    nc.scalar.activation(
                        dst, op, mybir.ActivationFunctionType.Relu,
                        bias=bias_sb[:, 1:2],
                    )
            if nb == g0 + gl - 1:
                for oh in range(OH):
                    q = nc.scalar if (nb == NB - 1 and oh == 1) else nc.sync
                    q.dma_start(
                        out=out_d[oh * P : (oh + 1) * P, g0 * P : (nb + 1) * P],
                        in_=osb[oh],
                    )

    nc.compile()
    return nc


def _get_nc():
    global _NC
    if _NC is None:
        _NC = _build_nc()
    return _NC


def _prep_inputs(x, adj, weight, bias):
    import ml_dtypes

    bf16 = ml_dtypes.bfloat16

    x = np.asarray(x, dtype=np.float32)
    adj = np.asarray(adj, dtype=np.float32)
    weight = np.ascontiguousarray(np.asarray(weight, dtype=np.float32))
    bias = np.ascontiguousarray(np.asarray(bias, dtype=np.float32))

    # xt[b] = x[b].T  -> [B, Din, N] bf16
    xt = x.transpose(0, 2, 1).astype(bf16)
    # adjt[b, nb, p, c*128+j] = adj[b, nb*128+j, c*128+p]
    a8 = adj.astype(bf16)
    adjt = (
        a8.reshape(B, NB, P, MB, P)
        .transpose(0, 1, 4, 3, 2)
        .reshape(B, NB, P, N)
    )
    in_maps = [
        {
            "xt": np.ascontiguousarray(xt[b]),
            "adjt": np.ascontiguousarray(adjt[b]),
            "weight": weight,
            "bias": bias,
        }
        for b in range(B)
    ]
    return in_maps


def kernel(x, adj, weight, bias, _trace=False):
    from concourse import bass_utils

    in_maps = _prep_inputs(x, adj, weight, bias)
    nc = _get_nc()
    res = bass_utils.run_bass_kernel_spmd(
        nc, in_maps, core_ids=list(range(B)), trace=_trace
    )
    # device output is outT = out.T in bf16; un-transpose + upcast on host
    out = np.stack(
        [np.asarray(r["out"], dtype=np.float32).T for r in res.results], axis=0
    )
    if _trace:
        return out, res
    return out


# revision 33
# speedup vs baseline: 1.0927x; 1.0927x over previous
"""Trainium2 Bass kernel for DiscreteGCNLayer — fp8 DoubleRowSwInterleave stage 2.

out = relu(adj @ (x @ ternary(w)) + bias).  Data-parallel over batch (8 cores).

Numerics (measured rel err 0.0124 vs the 2e-2 gate on the real inputs):
  adj     = 0.5 + a0,  a0 = e4m3(adj - 0.5)        (host)
  support = x @ dw  (bf16 stage-1) -> s0 = e4m3(support), s1 = e4m3(support-s0)
  out     = a0 @ s0 + a0 @ s1 + (0.5 * colsum(support) + bias) . 1^T
The a0 matmuls run in fp8 DoubleRowSwInterleave (0.5 cyc/row): a0 is the
stationary operand, host-packed per row block into the SwInterleave layout
(chunk pairs A/B interleaved per column, columns reversed); s0/s1 chunk pairs
are the moving operand in natural [128, 2, 128] slices.  adj traffic halves
to 4 MB/core and stage-2 PE time halves vs bf16.
"""

import sys

import numpy as np

if "/opt/trn_rl_repo" not in sys.path:
    sys.path.insert(0, "/opt/trn_rl_repo")

B = 8
N = 2048
DIN = 256
DOUT = 256
P = 128
NB = N // P   # 16 output row blocks
MB = N // P   # 16 contraction chunks (stage 2)
CP = MB // 2  # 8 chunk pairs (DoubleRow)
IB = DIN // P
OH = DOUT // P
SPARSITY = 0.01

_NC = None


def _build_nc():
    from contextlib import ExitStack

    import concourse.bass as bass  # noqa: F401
    import concourse.mybir as mybir
    import concourse.tile as tile
    from concourse import bacc

    F32 = mybir.dt.float32
    BF16 = mybir.dt.bfloat16
    FP8 = mybir.dt.float8e4
    Alu = mybir.AluOpType
    DRSW = mybir.MatmulPerfMode.DoubleRowSwInterleave

    nc = bacc.Bacc()
    xt_d = nc.dram_tensor("xt", [DIN, N], BF16, kind="ExternalInput")
    adj8_d = nc.dram_tensor("adj8", [NB, CP, P, 2 * P], FP8, kind="ExternalInput")
    w_d = nc.dram_tensor("weight", [DIN, DOUT], F32, kind="ExternalInput")
    b_d = nc.dram_tensor("bias", [DOUT], F32, kind="ExternalInput")
    out_d = nc.dram_tensor("out", [N, DOUT], BF16, kind="ExternalOutput")

    with tile.TileContext(nc) as tc, ExitStack() as ctx:
        singles = ctx.enter_context(tc.tile_pool(name="singles", bufs=1))
        out_pool = ctx.enter_context(tc.tile_pool(name="outsb", bufs=2))
        psum_s1 = ctx.enter_context(tc.tile_pool(name="ps1", bufs=6, space="PSUM"))
        psum_s2 = ctx.enter_context(tc.tile_pool(name="ps2", bufs=2, space="PSUM"))

        # --- DMA schedule (transfers serialize; every DMA >= 256 KB-ish to
        # stay above the ~650 ns per-transfer config pacing)
        w_sb = singles.tile([P, IB, DOUT], F32)
        nc.sync.dma_start(out=w_sb, in_=w_d[:].rearrange("(c p) o -> p c o", p=P))
        bias_row = singles.tile([1, DOUT], F32)
        nc.gpsimd.dma_start(out=bias_row, in_=b_d[:].rearrange("(p o) -> p o", p=1))

        xt_sb = singles.tile([P, IB, N], BF16)
        xt_r = xt_d[:].rearrange("(c p) m -> p c m", p=P)
        XQ = 4
        QW = N // XQ
        for q in range(XQ):
            nc.sync.dma_start(
                out=xt_sb[:, :, q * QW : (q + 1) * QW],
                in_=xt_r[:, :, q * QW : (q + 1) * QW],
            )

        adj8_sb = singles.tile([P, NB, CP, 2 * P], FP8)  # 32 KB/partition
        for nb in range(NB):  # 256 KB single-row-block slabs
            nc.sync.dma_start(
                out=adj8_sb[:, nb],
                in_=adj8_d[nb : nb + 1].rearrange("b c p f -> p b c f")[:, 0],
            )

        # --- PE warm-up burst + act-table hoist
        junk = singles.tile([P, 512], BF16)
        nc.vector.memset(junk, 1.0)
        actwarm = singles.tile([1, 8], BF16)
        nc.scalar.activation(
            actwarm, junk[0:1, 0:8], mybir.ActivationFunctionType.Relu
        )
        for wu in range(7):
            wt = psum_s2.tile([P, 512], F32, tag="s2")
            nc.tensor.matmul(wt, lhsT=junk[:, 0:P], rhs=junk, start=True, stop=True)

        ones8 = singles.tile([P, 2 * P], FP8)  # full-width: DRSW Ldweights
        nc.vector.memset(ones8, 1.0)       # needs >=128 active columns
        ones_bf = singles.tile([1, P], BF16)
        nc.vector.memset(ones_bf, 1.0)

        # ternary-quantized weight in bf16 (per i-chunk, pipelined with w DMA)
        dw_sb = singles.tile([P, IB, DOUT], BF16)
        tpos = singles.tile([P, IB, DOUT], F32)
        tneg = singles.tile([P, IB, DOUT], F32)
        for c in range(IB):
            nc.vector.tensor_scalar(
                out=tpos[:, c, :], in0=w_sb[:, c, :], scalar1=SPARSITY,
                scalar2=SPARSITY, op0=Alu.is_gt, op1=Alu.mult,
            )
            nc.vector.tensor_scalar(
                out=tneg[:, c, :], in0=w_sb[:, c, :], scalar1=-SPARSITY,
                scalar2=SPARSITY, op0=Alu.is_lt, op1=Alu.mult,
            )
            nc.vector.tensor_sub(dw_sb[:, c, :], tpos[:, c, :], tneg[:, c, :])

        # --- stage 1 (bf16): support chunks -> s0 = e4m3(psum) on ACT,
        # s1l = e4m3(psum - s0) on DVE
        s0_sb = singles.tile([P, MB, DOUT], FP8)
        s1_sb = singles.tile([P, MB, DOUT], FP8)
        for mb in range(MB):
            sp = psum_s1.tile([P, DOUT], F32, tag="s1")
            for c in range(IB):
                nc.tensor.matmul(
                    sp,
                    lhsT=xt_sb[:, c, mb * P : (mb + 1) * P],
                    rhs=dw_sb[:, c, :],
                    start=(c == 0),
                    stop=(c == IB - 1),
                )
            nc.scalar.copy(s0_sb[:, mb, :], sp)
            nc.vector.tensor_tensor(
                out=s1_sb[:, mb, :], in0=sp, in1=s0_sb[:, mb, :],
                op=Alu.subtract,
            )

        # --- colsum (DoubleRow rank-2): colsum[o] = sum_m support[m, o]
        cs = psum_s2.tile([P, DOUT], F32, tag="s2", name="cs")
        for src in (s0_sb, s1_sb):
            for cp in range(CP):
                nc.tensor.matmul(
                    cs,
                    lhsT=ones8,
                    rhs=src[:, 2 * cp : 2 * cp + 2, :],
                    start=(src is s0_sb and cp == 0),
                    stop=(src is s1_sb and cp == CP - 1),
                    perf_mode=DRSW,
                )
        cb = singles.tile([1, DOUT], BF16)
        nc.vector.scalar_tensor_tensor(
            out=cb, in0=cs[0:1, :], scalar=0.5, in1=bias_row,
            op0=Alu.mult, op1=Alu.add,
        )

        # --- stage 2: out[nb][n, oh] = relu( sum_cp a0[nb,cp]^T (.) (s0|s1)
        #             + ones^T cb )   in fp8 DoubleRowSwInterleave
        GRPS = [(0, 14), (14, 2)]
        grp_of = {}
        for g in GRPS:
            for nb in range(g[0], g[0] + g[1]):
                grp_of[nb] = g
        osb = None
        for nb in range(NB):
            g0, gl = grp_of[nb]
            if nb == g0:
                osb = out_pool.tile(
                    [P, gl, DOUT], BF16, tag="o", name=f"osb_{nb}", bufs=2
                )
            for oh in range(OH):
                op = psum_s2.tile([P, P], F32, tag="s2")
                for src in (s0_sb, s1_sb):
                    for cp in range(CP):
                        nc.tensor.matmul(
                            op,
                            lhsT=adj8_sb[:, nb, cp, :],
                            rhs=src[:, 2 * cp : 2 * cp + 2, oh * P : (oh + 1) * P],
                            start=(src is s0_sb and cp == 0),
                            stop=False,
                            perf_mode=DRSW,
                        )
                nc.tensor.matmul(
                    op, lhsT=ones_bf, rhs=cb[:, oh * P : (oh + 1) * P],
                    start=False, stop=True,
                )
                dst = osb[:, nb - g0, oh * P : (oh + 1) * P]
                if oh == 0:
                    nc.vector.tensor_scalar(
                        out=dst, in0=op, scalar1=0.0, scalar2=None, op0=Alu.max
                    )
                else:
                    nc.scalar.activation(
                        dst, op, mybir.ActivationFunctionType.Relu
                    )
            if nb == g0 + gl - 1:
                nc.sync.dma_start(
                    out=out_d[g0 * P : (nb + 1) * P].rearrange(
                        "(c p) o -> p c o", p=P
                    ),
                    in_=osb,
                )

    nc.compile()
    return nc


def _get_nc():
    global _NC
    if _NC is None:
        _NC = _build_nc()
    return _NC


def _prep_inputs(x, adj, weight, bias):
    import ml_dtypes

    bf16 = ml_dtypes.bfloat16
    e4m3 = ml_dtypes.float8_e4m3

    x = np.asarray(x, dtype=np.float32)
    adj = np.asarray(adj, dtype=np.float32)
    weight = np.ascontiguousarray(np.asarray(weight, dtype=np.float32))
    bias = np.ascontiguousarray(np.asarray(bias, dtype=np.float32))

    xt = x.transpose(0, 2, 1).astype(bf16)
    # a0 packed for SwInterleave: per (nb, chunk-pair) a [128, 256] lhsT whose
    # rows read [A_{127}, B_{127}, ..., A_0, B_0] with
    # A[k, n] = a0[nb*128+n, (2cp)*128+k], B likewise for 2cp+1.
    a0 = (adj - 0.5).astype(e4m3)
    t = a0.reshape(B, NB, P, CP, 2, P)       # b, nb, n, cp, par, k
    t = t[:, :, ::-1, :, :, :]               # reverse n -> j
    pk = t.transpose(0, 1, 3, 5, 2, 4)       # b, nb, cp, k, j, par
    adj8 = np.ascontiguousarray(pk.reshape(B, NB, CP, P, 2 * P))

    in_maps = [
        {
            "xt": np.ascontiguousarray(xt[b]),
            "adj8": adj8[b],
            "weight": weight,
            "bias": bias,
        }
        for b in range(B)
    ]
    return in_maps


def kernel(x, adj, weight, bias, _trace=False):
    from concourse import bass_utils

    in_maps = _prep_inputs(x, adj, weight, bias)
    nc = _get_nc()
    res = bass_utils.run_bass_kernel_spmd(
        nc, in_maps, core_ids=list(range(B)), trace=_trace
    )
    out = np.stack(
        [np.asarray(r["out"], dtype=np.float32) for r in res.results], axis=0
    )
    if _trace:
        return out, res
    return out
